# revision 13
# baseline (speedup 1.0000x reference)
"""TRN2 Bass kernel: masked multi-head attention block (B=4, S=2048, C=768, H=12).

Sharding: 8 cores = 4 batches x 2 head-groups (6 heads each).  Each core runs a
flash-attention-style Bass/Tile kernel over its (batch, head-group) shard.

v2: full-fp16 matmul datapath (fp16 streams at 1 cyc/col on the PE like bf16,
enables fast-weight-load, and halves DVE/DMA cost vs fp32) with the mask
applied as a post-exp elementwise multiply on the vector engine (fp16 2x mode)
instead of a pre-exp fp32 add from PSUM (1x mode):

  qT/kT: [384, S] feature-major fp16 projections from xT (q pre-scaled)
  v:     [S, 6*65] fp16, a ones column appended per head (softmax denominator)
  scoresT[k, q] = k . q (contract hd=64, head pairs row-packed on the PE array)
  pT = exp(scoresT) on the scalar engine (psum -> sbuf fp16)
  pT *= keepT (0/1 mask) on the vector engine (fp16 2x)
  avT[65, 512] accumulated over key chunks; row 64 = softmax denominator
  attn_outT = avT[0:64] * recip(denominator)  (partition-broadcast on gpsimd)
  y_partial = attn_outT.T @ w_projT slice  (row-parallel output projection)

Host-side: transposes/slices the weights per core into fp16, converts the mask
to a 0/1 fp16 keep-mask, sums the two per-batch partials, and adds b_proj.
"""

from contextlib import ExitStack

import numpy as np

import concourse.tile as tile
from concourse import bacc, mybir
from concourse.bass_utils import run_bass_kernel_spmd

F32 = mybir.dt.float32
F16 = mybir.dt.float16
I16 = mybir.dt.int16

# fp16 Schraudolph exp: fp16_bits(exp(s)) ~= round(s*1024*log2(e) + B).
# B = 15*1024 - 59 centers the (1+x)/2^x mantissa error (rms 1.8%, max 3.9%);
# used on a small fraction of groups to offload the scalar engine.
SCHRAU_A = 1477.3195458351
SCHRAU_B = 15301.0

B, S, C, H = 4, 2048, 768, 12
HD = 64
H_PER_CORE = 6
D_CORE = H_PER_CORE * HD  # 384
QBLK = 512
GRP = 2
N_CORES = 8


def _build_kernel():
    nc = bacc.Bacc(
        trn_type="TRN2", target_bir_lowering=False, debug=False, num_devices=N_CORES
    )
    KC = S // 128  # 16 key chunks
    QB = S // QBLK  # 4 query blocks
    NB = S // QBLK  # 4 column chunks for projections
    ST = S // 128  # 16 sequence tiles
    groups = [(2 * i, GRP) for i in range(KC // GRP)]  # 8 groups of 2 key chunks

    xT = nc.dram_tensor("xT", [C, S], F16, kind="ExternalInput").ap()
    wq = nc.dram_tensor("wq", [C, D_CORE], F16, kind="ExternalInput").ap()
    wk = nc.dram_tensor("wk", [C, D_CORE], F16, kind="ExternalInput").ap()
    wv = nc.dram_tensor("wv", [C, D_CORE], F16, kind="ExternalInput").ap()
    wproj = nc.dram_tensor("wproj", [D_CORE, C], F16, kind="ExternalInput").ap()
    vones = nc.dram_tensor("vones", [128, S // 128 * H_PER_CORE], F16, kind="ExternalInput").ap()
    keepT = nc.dram_tensor("keepT", [S, S], F16, kind="ExternalInput").ap()
    y = nc.dram_tensor("y", [S, C], F32, kind="ExternalOutput").ap()

    with tile.TileContext(nc) as tc, ExitStack() as ctx:
        consts = ctx.enter_context(tc.tile_pool(name="consts", bufs=1))
        qkv_pool = ctx.enter_context(tc.tile_pool(name="qkv", bufs=1))
        mpool = ctx.enter_context(tc.tile_pool(name="mask", bufs=4))
        ypool = ctx.enter_context(tc.tile_pool(name="y", bufs=4))

        keepT_r = keepT.rearrange("(kc p) q -> p kc q", p=128)
        mask_cache = {}

        def load_mask(qb_i):
            halves = []
            for half in range(KC // 8):
                mh = mpool.tile([128, 8, QBLK], F16, tag="mask", name="mask_h")
                nc.sync.dma_start(
                    mh[:],
                    keepT_r[
                        :,
                        half * 8 : (half + 1) * 8,
                        qb_i * QBLK : (qb_i + 1) * QBLK,
                    ],
                )
                halves.append(mh)
            return halves

        qT_sb = qkv_pool.tile([128, 3, S], F16)
        kT_sb = qkv_pool.tile([128, 3, S], F16)
        vaug_sb = qkv_pool.tile([128, ST, H_PER_CORE * (HD + 1)], F16)
        attn_sb = qkv_pool.tile([128, 3, S], F16)

        # ---------------- phase 1: qkv projections ----------------
        with ExitStack() as p1:
            wpool = p1.enter_context(tc.tile_pool(name="w1", bufs=1))
            xpool = p1.enter_context(tc.tile_pool(name="x1", bufs=1))
            ps1 = p1.enter_context(tc.tile_pool(name="ps1", bufs=3, space="PSUM"))
            psv1 = p1.enter_context(tc.tile_pool(name="psv1", bufs=2, space="PSUM"))

            wk_sb = wpool.tile([128, 6, D_CORE], F16)
            wv_sb = wpool.tile([128, 6, D_CORE], F16)
            wq_sb = wpool.tile([128, 6, D_CORE], F16)
            xT_sb = xpool.tile([128, 6, S], F16)
            xT_r = xT.rearrange("(t p) s -> p t s", p=128)

            def x_chunk_dma(nb):
                nc.sync.dma_start(
                    xT_sb[:, :, nb * QBLK : (nb + 1) * QBLK],
                    xT_r[:, :, nb * QBLK : (nb + 1) * QBLK],
                )

            # interleave weight and x-chunk DMAs in consumption order so the
            # first projection matmuls start as early as possible
            nc.sync.dma_start(wk_sb[:], wk.rearrange("(t p) d -> p t d", p=128))
            x_chunk_dma(0)
            nc.sync.dma_start(wv_sb[:], wv.rearrange("(t p) d -> p t d", p=128))
            x_chunk_dma(1)
            x_chunk_dma(2)
            x_chunk_dma(3)
            nc.sync.dma_start(wq_sb[:], wq.rearrange("(t p) d -> p t d", p=128))

            # warm the ACT exp table during input DMA so the first real exp
            # doesn't pay the ~2.7us table-load
            dummy = consts.tile([1, 8], F32)
            nc.scalar.activation(dummy[:], wk_sb[0:1, 0, 0:8], mybir.ActivationFunctionType.Exp)

            # non-critical input DMAs after the phase-1 ones
            wproj_sb = consts.tile([128, 3, C], F16)
            nc.sync.dma_start(wproj_sb[:], wproj.rearrange("(t p) o -> p t o", p=128))
            # ones columns (softmax denominator) come from DRAM
            vaug_ones = vaug_sb.rearrange("p st (h u) -> p st h u", u=HD + 1)[:, :, :, HD]
            nc.sync.dma_start(
                vaug_ones, vones.rearrange("p (st h) -> p st h", h=H_PER_CORE)
            )
            mask_cache[0] = load_mask(0)
            mask_cache[1] = load_mask(1)

            # k projection first (phase 2 needs all of kT), then v, then q
            def qk_proj(w_sb, dst):
                for nb in range(NB):
                    for m in range(3):
                        ps = ps1.tile([128, QBLK], F32, tag="psqk", name="psqk")
                        for k in range(6):
                            nc.tensor.matmul(
                                ps[:],
                                w_sb[:, k, m * 128 : (m + 1) * 128],
                                xT_sb[:, k, nb * QBLK : (nb + 1) * QBLK],
                                start=(k == 0),
                                stop=(k == 5),
                            )
                        nc.vector.tensor_copy(
                            dst[:, m, nb * QBLK : (nb + 1) * QBLK], ps[:]
                        )

            qk_proj(wk_sb, kT_sb)
            for st in range(ST):
                psv = psv1.tile([128, D_CORE], F32, tag="psv", name="psv")
                for k in range(6):
                    nc.tensor.matmul(
                        psv[:],
                        xT_sb[:, k, st * 128 : (st + 1) * 128],
                        wv_sb[:, k, :],
                        start=(k == 0),
                        stop=(k == 5),
                    )
                # scatter the 6 heads into the augmented-v layout in one op
                nc.vector.tensor_copy(
                    vaug_sb.rearrange("p st (h u) -> p st h u", u=HD + 1)[
                        :, st, :, 0:HD
                    ],
                    psv.rearrange("p (h d) -> p h d", d=HD)[:],
                )
            qk_proj(wq_sb, qT_sb)

        # ---------------- phase 2: attention (+ interleaved projection) ----
        with ExitStack() as p2:
            ppool = p2.enter_context(tc.tile_pool(name="pT", bufs=7))
            dpool = p2.enter_context(tc.tile_pool(name="div", bufs=3))
            bpool = p2.enter_context(tc.tile_pool(name="bcast", bufs=3))
            apool = p2.enter_context(tc.tile_pool(name="avsb", bufs=2))
            ps_s = p2.enter_context(tc.tile_pool(name="ps_s", bufs=3, space="PSUM"))
            ps_av = p2.enter_context(tc.tile_pool(name="ps_av", bufs=2, space="PSUM"))

            y_r = y.rearrange("(st p) o -> st p o", p=128)

            def proj(qb):
                # output projection for the 4 sequence tiles of query block qb
                for st in range(4 * qb, 4 * qb + 4):
                    y_sb = ypool.tile([128, C], F32, tag="ysb", name="y_sb")
                    for nb2 in range(2):
                        ps = ps_av.tile([128, 384], F32, tag="av", name="psy")
                        for k3 in range(3):
                            nc.tensor.matmul(
                                ps[:],
                                attn_sb[:, k3, st * 128 : (st + 1) * 128],
                                wproj_sb[:, k3, nb2 * 384 : (nb2 + 1) * 384],
                                start=(k3 == 0),
                                stop=(k3 == 2),
                            )
                        if nb2 == 0:
                            nc.vector.tensor_copy(y_sb[:, :384], ps[:])
                        else:
                            nc.scalar.copy(y_sb[:, 384:], ps[:])
                    nc.sync.dma_start(y_r[st], y_sb[:])

            for qb in range(QB):
                if qb + 2 < QB:
                    mask_cache[qb + 2] = load_mask(qb + 2)
                mask_halves = mask_cache.pop(qb)
                av_all = apool.tile([HD + 1, H_PER_CORE, QBLK], F32, tag="av_all", name="av_all")

                for hp in range(3):
                    hA, hB = 2 * hp, 2 * hp + 1
                    av = [
                        ps_av.tile([HD + 1, QBLK], F32, tag="av", name=f"av{hp}a"),
                        ps_av.tile([HD + 1, QBLK], F32, tag="av", name=f"av{hp}b"),
                    ]
                    for (g0, gs) in groups:
                        mh = mask_halves[g0 // 8]
                        moff = g0 % 8
                        sc = [
                            ps_s.tile([128, GRP, QBLK], F32, tag="sc", name="scA"),
                            ps_s.tile([128, GRP, QBLK], F32, tag="sc", name="scB"),
                        ]
                        # interleave heads so the row-packed matmul pairs
                        # overlap on the PE array (different row groups)
                        for c in range(gs):
                            kc = g0 + c
                            for i, h in ((0, hA), (1, hB)):
                                row0 = (h % 2) * HD
                                nc.tensor.matmul(
                                    sc[i][:, c, :],
                                    kT_sb[
                                        row0 : row0 + HD,
                                        h // 2,
                                        kc * 128 : (kc + 1) * 128,
                                    ],
                                    qT_sb[
                                        row0 : row0 + HD,
                                        h // 2,
                                        qb * QBLK : (qb + 1) * QBLK,
                                    ],
                                    start=True,
                                    stop=True,
                                    tile_position=(row0, 0),
                                )
                        pT = ppool.tile([128, 2, GRP, QBLK], F16, tag="pT", name="pT")
                        for i in range(2):
                            nc.scalar.activation(
                                pT[:, i, :gs, :],
                                sc[i][:, :gs, :],
                                mybir.ActivationFunctionType.Exp,
                            )
                        # apply 0/1 keep-mask to both heads in one DVE op
                        # (fp16 2x mode, mask broadcast over the head axis)
                        nc.vector.tensor_mul(
                            pT[:, :, :gs, :],
                            pT[:, :, :gs, :],
                            mh[:, moff : moff + gs, :].unsqueeze(1).broadcast_to(
                                [128, 2, gs, QBLK]
                            ),
                        )
                        for i, h in ((0, hA), (1, hB)):
                            for c in range(gs):
                                kc = g0 + c
                                nc.tensor.matmul(
                                    av[i][:],
                                    vaug_sb[:, kc, h * (HD + 1) : (h + 1) * (HD + 1)],
                                    pT[:, i, c, :],
                                    start=(kc == 0),
                                    stop=(kc == KC - 1),
                                )
                    nc.vector.tensor_copy(av_all[:, hA, :], av[0][:])
                    nc.scalar.copy(av_all[:, hB, :], av[1][:])

                    # normalize this head pair now (pipelines behind the next
                    # pair's matmul streak instead of serializing at qb end)
                    dstack = dpool.tile([2, QBLK], F32, tag="dstack", name="dstack")
                    nc.gpsimd.dma_start(dstack[:], av_all[HD : HD + 1, hA : hB + 1, :])
                    recip = dpool.tile([2, QBLK], F32, tag="recip", name="recip")
                    nc.vector.reciprocal_approx_fast(recip[:], dstack[:])
                    r2 = bpool.tile([1, 2, QBLK], F32, tag="r2", name="r2")
                    nc.gpsimd.dma_start(r2[:], recip[:])
                    for i, h in ((0, hA), (1, hB)):
                        bc = bpool.tile([HD, QBLK], F32, tag="bc", name="bc")
                        nc.gpsimd.partition_broadcast(bc[:], r2[:, i, :])
                        if i == 0:
                            dst = attn_sb[:HD, hp, qb * QBLK : (qb + 1) * QBLK]
                            nc.vector.tensor_mul(dst, av_all[:HD, h, :], bc[:])
                        else:
                            tmp = bpool.tile([HD, QBLK], F16, tag="tmpo", name="tmpo")
                            nc.vector.tensor_mul(tmp[:], av_all[:HD, h, :], bc[:])
                            nc.gpsimd.dma_start(
                                attn_sb[HD:128, hp, qb * QBLK : (qb + 1) * QBLK],
                                tmp[:],
                            )
                # project the previous query block while this one's softmax
                # normalization completes (keeps the PE busy, spreads y DMA)
                if qb >= 1:
                    proj(qb - 1)
            proj(QB - 1)

    nc.compile()
    return nc


def _prep_core_inputs(x, mask, w_qkv, w_proj, core):
    b, g = core // 2, core % 2
    scale = HD ** -0.5
    s0, s1 = 384 * g, 384 * (g + 1)
    return {
        "xT": np.ascontiguousarray(x[b].T).astype(np.float16),
        "wq": np.ascontiguousarray((w_qkv[s0:s1, :] * scale).T).astype(np.float16),
        "wk": np.ascontiguousarray(w_qkv[C + s0 : C + s1, :].T).astype(np.float16),
        "wv": np.ascontiguousarray(w_qkv[2 * C + s0 : 2 * C + s1, :].T).astype(np.float16),
        "wproj": np.ascontiguousarray(w_proj[:, s0:s1].T).astype(np.float16),
        "keepT": np.ascontiguousarray((1 - mask[b]).T).astype(np.float16),
        "vones": np.ones((128, S // 128 * H_PER_CORE), dtype=np.float16),
    }


_NC_CACHE = {}


def get_nc():
    if "nc" not in _NC_CACHE:
        _NC_CACHE["nc"] = _build_kernel()
    return _NC_CACHE["nc"]


def _build_runner(nc):
    """Reusable jitted shard_map callable over the 8 cores (mirrors
    bass2jax.run_bass_via_pjrt but cacheable across calls)."""
    import jax
    from jax.experimental.shard_map import shard_map
    from jax.sharding import Mesh, PartitionSpec

    from concourse.bass2jax import (
        _bass_exec_p,
        install_neuronx_cc_hook,
        partition_id_tensor,
    )

    install_neuronx_cc_hook()
    partition_name = nc.partition_id_tensor.name if nc.partition_id_tensor else None
    in_names, out_names, out_avals, zero_outs = [], [], [], []
    for alloc in nc.m.functions[0].allocations:
        if not isinstance(alloc, mybir.MemoryLocationSet):
            continue
        name = alloc.memorylocations[0].name
        if alloc.kind == "ExternalInput":
            if name != partition_name:
                in_names.append(name)
        elif alloc.kind == "ExternalOutput":
            out_names.append(name)
            shape = tuple(alloc.tensor_shape)
            dtype = mybir.dt.np(alloc.dtype)
            out_avals.append(jax.core.ShapedArray(shape, dtype))
            zero_outs.append(np.zeros(shape, dtype))
    n_params = len(in_names)
    all_in_names = list(in_names) + list(out_names)
    if partition_name is not None:
        all_in_names.append(partition_name)

    def _body(*args):
        operands = list(args)
        if partition_name is not None:
            operands.append(partition_id_tensor())
        outs = _bass_exec_p.bind(
            *operands,
            out_avals=tuple(out_avals),
            in_names=tuple(all_in_names),
            out_names=tuple(out_names),
            lowering_input_output_aliases=(),
            sim_require_finite=True,
            sim_require_nnan=True,
            nc=nc,
        )
        return tuple(outs)

    n_cores = nc.num_devices
    devices = jax.devices()[:n_cores]
    mesh = Mesh(np.asarray(devices), ("core",))
    in_specs = (PartitionSpec("core"),) * (n_params + len(out_names))
    out_specs = (PartitionSpec("core"),) * len(out_names)
    fn = jax.jit(
        shard_map(
            _body, mesh=mesh, in_specs=in_specs, out_specs=out_specs, check_rep=False
        ),
        keep_unused=True,
    )
    return fn, in_names, out_names, zero_outs


_RUNNER_CACHE = {}


def get_runner(nc, in_maps):
    """Return (fn, dev_args) for repeated dispatch of `nc` with `in_maps`."""
    import jax
    from jax.sharding import Mesh, NamedSharding, PartitionSpec

    key = id(nc)
    if key not in _RUNNER_CACHE:
        _RUNNER_CACHE[key] = _build_runner(nc)
    fn, in_names, out_names, zero_outs = _RUNNER_CACHE[key]
    n_cores = nc.num_devices
    mesh = Mesh(np.asarray(jax.devices()[:n_cores]), ("core",))
    shard = NamedSharding(mesh, PartitionSpec("core"))
    concat_in = [
        np.concatenate([np.asarray(in_maps[c][n]) for c in range(n_cores)], axis=0)
        for n in in_names
    ]
    dev_in = [jax.device_put(a, shard) for a in concat_in]
    zkey = ("zeros", key)
    if zkey not in _RUNNER_CACHE:
        concat_zeros = [
            np.zeros((n_cores * z.shape[0], *z.shape[1:]), z.dtype) for z in zero_outs
        ]
        _RUNNER_CACHE[zkey] = [jax.device_put(a, shard) for a in concat_zeros]
    return fn, dev_in + _RUNNER_CACHE[zkey]


def run_cached(nc, in_maps):
    """Execute via the cached runner; returns per-core result dicts."""
    fn, dev_args = get_runner(nc, in_maps)
    out_arrs = fn(*dev_args)
    _, _, out_names, zero_outs = _RUNNER_CACHE[id(nc)]
    n_cores = nc.num_devices
    fetched = [
        np.asarray(a).reshape(n_cores, *zero_outs[i].shape)
        for i, a in enumerate(out_arrs)
    ]
    return [
        {name: fetched[i][c] for i, name in enumerate(out_names)}
        for c in range(n_cores)
    ]


def make_in_maps(x, mask, w_qkv, w_proj):
    return [_prep_core_inputs(x, mask, w_qkv, w_proj, c) for c in range(N_CORES)]


def combine(results, b_proj):
    outs = []
    for b in range(B):
        outs.append(results[2 * b]["y"] + results[2 * b + 1]["y"] + b_proj[None, :])
    return np.stack(outs).astype(np.float32)


def kernel(x, mask, w_qkv, w_proj, b_proj):
    x = np.asarray(x, dtype=np.float32)
    mask = np.asarray(mask)
    w_qkv = np.asarray(w_qkv, dtype=np.float32)
    w_proj = np.asarray(w_proj, dtype=np.float32)
    b_proj = np.asarray(b_proj, dtype=np.float32)

    nc = get_nc()
    in_maps = make_in_maps(x, mask, w_qkv, w_proj)
    try:
        results = run_cached(nc, in_maps)
    except Exception:
        results = run_bass_kernel_spmd(nc, in_maps, list(range(N_CORES))).results
    return combine(results, b_proj)


# revision 14
# speedup vs baseline: 1.0047x; 1.0047x over previous
"""TRN2 Bass kernel: masked multi-head attention block (B=4, S=2048, C=768, H=12).

Sharding: 8 cores = 4 batches x 2 head-groups (6 heads each).  Each core runs a
flash-attention-style Bass/Tile kernel over its (batch, head-group) shard.

v2: full-fp16 matmul datapath (fp16 streams at 1 cyc/col on the PE like bf16,
enables fast-weight-load, and halves DVE/DMA cost vs fp32) with the mask
applied as a post-exp elementwise multiply on the vector engine (fp16 2x mode)
instead of a pre-exp fp32 add from PSUM (1x mode):

  qT/kT: [384, S] feature-major fp16 projections from xT (q pre-scaled)
  v:     [S, 6*65] fp16, a ones column appended per head (softmax denominator)
  scoresT[k, q] = k . q (contract hd=64, head pairs row-packed on the PE array)
  pT = exp(scoresT) on the scalar engine (psum -> sbuf fp16)
  pT *= keepT (0/1 mask) on the vector engine (fp16 2x)
  avT[65, 512] accumulated over key chunks; row 64 = softmax denominator
  attn_outT = avT[0:64] * recip(denominator)  (partition-broadcast on gpsimd)
  y_partial = attn_outT.T @ w_projT slice  (row-parallel output projection)

Host-side: transposes/slices the weights per core into fp16, converts the mask
to a 0/1 fp16 keep-mask, sums the two per-batch partials, and adds b_proj.
"""

from contextlib import ExitStack

import numpy as np

import concourse.tile as tile
from concourse import bacc, mybir
from concourse.bass_utils import run_bass_kernel_spmd

F32 = mybir.dt.float32
F16 = mybir.dt.float16
I16 = mybir.dt.int16

# fp16 Schraudolph exp: fp16_bits(exp(s)) ~= round(s*1024*log2(e) + B).
# B = 15*1024 - 59 centers the (1+x)/2^x mantissa error (rms 1.8%, max 3.9%);
# used on a small fraction of groups to offload the scalar engine.
SCHRAU_A = 1477.3195458351
SCHRAU_B = 15301.0

B, S, C, H = 4, 2048, 768, 12
HD = 64
H_PER_CORE = 6
D_CORE = H_PER_CORE * HD  # 384
QBLK = 512
GRP = 2
N_CORES = 8


def _build_kernel():
    nc = bacc.Bacc(
        trn_type="TRN2", target_bir_lowering=False, debug=False, num_devices=N_CORES
    )
    KC = S // 128  # 16 key chunks
    QB = S // QBLK  # 4 query blocks
    NB = S // QBLK  # 4 column chunks for projections
    ST = S // 128  # 16 sequence tiles
    groups = [(2 * i, GRP) for i in range(KC // GRP)]  # 8 groups of 2 key chunks

    xT = nc.dram_tensor("xT", [C, S], F16, kind="ExternalInput").ap()
    wq = nc.dram_tensor("wq", [C, D_CORE], F16, kind="ExternalInput").ap()
    wk = nc.dram_tensor("wk", [C, D_CORE], F16, kind="ExternalInput").ap()
    wv = nc.dram_tensor("wv", [C, D_CORE], F16, kind="ExternalInput").ap()
    wproj = nc.dram_tensor("wproj", [D_CORE, C], F16, kind="ExternalInput").ap()
    vones = nc.dram_tensor("vones", [128, S // 128 * H_PER_CORE], F16, kind="ExternalInput").ap()
    keepT = nc.dram_tensor("keepT", [S, S], F16, kind="ExternalInput").ap()
    y = nc.dram_tensor("y", [S, C], F32, kind="ExternalOutput").ap()

    with tile.TileContext(nc) as tc, ExitStack() as ctx:
        consts = ctx.enter_context(tc.tile_pool(name="consts", bufs=1))
        qkv_pool = ctx.enter_context(tc.tile_pool(name="qkv", bufs=1))
        mpool = ctx.enter_context(tc.tile_pool(name="mask", bufs=4))
        ypool = ctx.enter_context(tc.tile_pool(name="y", bufs=3))

        keepT_r = keepT.rearrange("(kc p) q -> p kc q", p=128)
        mask_cache = {}

        def load_mask(qb_i):
            halves = []
            for half in range(KC // 8):
                mh = mpool.tile([128, 8, QBLK], F16, tag="mask", name="mask_h")
                nc.sync.dma_start(
                    mh[:],
                    keepT_r[
                        :,
                        half * 8 : (half + 1) * 8,
                        qb_i * QBLK : (qb_i + 1) * QBLK,
                    ],
                )
                halves.append(mh)
            return halves

        qT_sb = qkv_pool.tile([128, 3, S], F16)
        kT_sb = qkv_pool.tile([128, 3, S], F16)
        vaug_sb = qkv_pool.tile([128, ST, H_PER_CORE * (HD + 1)], F16)
        attn_sb = qkv_pool.tile([128, 3, S], F16)

        # ---------------- phase 1: qkv projections ----------------
        with ExitStack() as p1:
            wpool = p1.enter_context(tc.tile_pool(name="w1", bufs=1))
            xpool = p1.enter_context(tc.tile_pool(name="x1", bufs=1))
            ps1 = p1.enter_context(tc.tile_pool(name="ps1", bufs=3, space="PSUM"))
            psv1 = p1.enter_context(tc.tile_pool(name="psv1", bufs=2, space="PSUM"))

            wk_sb = wpool.tile([128, 6, D_CORE], F16)
            wv_sb = wpool.tile([128, 6, D_CORE], F16)
            wq_sb = wpool.tile([128, 6, D_CORE], F16)
            xT_sb = xpool.tile([128, 6, S], F16)
            xT_r = xT.rearrange("(t p) s -> p t s", p=128)

            def x_chunk_dma(nb):
                nc.sync.dma_start(
                    xT_sb[:, :, nb * QBLK : (nb + 1) * QBLK],
                    xT_r[:, :, nb * QBLK : (nb + 1) * QBLK],
                )

            # interleave weight and x-chunk DMAs in consumption order so the
            # first projection matmuls start as early as possible
            nc.sync.dma_start(wk_sb[:], wk.rearrange("(t p) d -> p t d", p=128))
            x_chunk_dma(0)
            nc.sync.dma_start(wv_sb[:], wv.rearrange("(t p) d -> p t d", p=128))
            x_chunk_dma(1)
            x_chunk_dma(2)
            x_chunk_dma(3)
            nc.sync.dma_start(wq_sb[:], wq.rearrange("(t p) d -> p t d", p=128))

            # warm the ACT exp table during input DMA so the first real exp
            # doesn't pay the ~2.7us table-load
            dummy = consts.tile([1, 8], F32)
            nc.scalar.activation(dummy[:], wk_sb[0:1, 0, 0:8], mybir.ActivationFunctionType.Exp)

            # non-critical input DMAs after the phase-1 ones
            wproj_sb = consts.tile([128, 3, C], F16)
            nc.sync.dma_start(wproj_sb[:], wproj.rearrange("(t p) o -> p t o", p=128))
            # ones columns (softmax denominator) come from DRAM
            vaug_ones = vaug_sb.rearrange("p st (h u) -> p st h u", u=HD + 1)[:, :, :, HD]
            nc.sync.dma_start(
                vaug_ones, vones.rearrange("p (st h) -> p st h", h=H_PER_CORE)
            )
            mask_cache[0] = load_mask(0)
            mask_cache[1] = load_mask(1)

            # k projection first (phase 2 needs all of kT), then v, then q
            def qk_proj(w_sb, dst):
                for nb in range(NB):
                    for m in range(3):
                        ps = ps1.tile([128, QBLK], F32, tag="psqk", name="psqk")
                        for k in range(6):
                            nc.tensor.matmul(
                                ps[:],
                                w_sb[:, k, m * 128 : (m + 1) * 128],
                                xT_sb[:, k, nb * QBLK : (nb + 1) * QBLK],
                                start=(k == 0),
                                stop=(k == 5),
                            )
                        nc.vector.tensor_copy(
                            dst[:, m, nb * QBLK : (nb + 1) * QBLK], ps[:]
                        )

            qk_proj(wk_sb, kT_sb)
            for st in range(ST):
                psv = psv1.tile([128, D_CORE], F32, tag="psv", name="psv")
                for k in range(6):
                    nc.tensor.matmul(
                        psv[:],
                        xT_sb[:, k, st * 128 : (st + 1) * 128],
                        wv_sb[:, k, :],
                        start=(k == 0),
                        stop=(k == 5),
                    )
                # scatter the 6 heads into the augmented-v layout in one op
                nc.vector.tensor_copy(
                    vaug_sb.rearrange("p st (h u) -> p st h u", u=HD + 1)[
                        :, st, :, 0:HD
                    ],
                    psv.rearrange("p (h d) -> p h d", d=HD)[:],
                )
            qk_proj(wq_sb, qT_sb)

        # ---------------- phase 2: attention (+ interleaved projection) ----
        with ExitStack() as p2:
            ppool = p2.enter_context(tc.tile_pool(name="pT", bufs=5))
            dpool = p2.enter_context(tc.tile_pool(name="div", bufs=2))
            bpool = p2.enter_context(tc.tile_pool(name="bcast", bufs=2))
            apool = p2.enter_context(tc.tile_pool(name="avsb", bufs=1))
            ps_s = p2.enter_context(tc.tile_pool(name="ps_s", bufs=3, space="PSUM"))
            ps_av = p2.enter_context(tc.tile_pool(name="ps_av", bufs=2, space="PSUM"))

            y_r = y.rearrange("(st p) o -> st p o", p=128)

            def proj(qb):
                # output projection for the 4 sequence tiles of query block qb
                for st in range(4 * qb, 4 * qb + 4):
                    y_sb = ypool.tile([128, C], F32, tag="ysb", name="y_sb")
                    for nb2 in range(2):
                        ps = ps_av.tile([128, 384], F32, tag="av", name="psy")
                        for k3 in range(3):
                            nc.tensor.matmul(
                                ps[:],
                                attn_sb[:, k3, st * 128 : (st + 1) * 128],
                                wproj_sb[:, k3, nb2 * 384 : (nb2 + 1) * 384],
                                start=(k3 == 0),
                                stop=(k3 == 2),
                            )
                        if nb2 == 0:
                            nc.vector.tensor_copy(y_sb[:, :384], ps[:])
                        else:
                            nc.scalar.copy(y_sb[:, 384:], ps[:])
                    nc.sync.dma_start(y_r[st], y_sb[:])

            for qb in range(QB):
                if qb + 2 < QB:
                    mask_cache[qb + 2] = load_mask(qb + 2)
                mask_halves = mask_cache.pop(qb)
                av_all = apool.tile([HD + 1, H_PER_CORE, QBLK], F32, tag="av_all", name="av_all")

                for hp in range(3):
                    hA, hB = 2 * hp, 2 * hp + 1
                    av = [
                        ps_av.tile([HD + 1, QBLK], F32, tag="av", name=f"av{hp}a"),
                        ps_av.tile([HD + 1, QBLK], F32, tag="av", name=f"av{hp}b"),
                    ]
                    for (g0, gs) in groups:
                        mh = mask_halves[g0 // 8]
                        moff = g0 % 8
                        sc = [
                            ps_s.tile([128, GRP, QBLK], F32, tag="sc", name="scA"),
                            ps_s.tile([128, GRP, QBLK], F32, tag="sc", name="scB"),
                        ]
                        # interleave heads so the row-packed matmul pairs
                        # overlap on the PE array (different row groups)
                        for c in range(gs):
                            kc = g0 + c
                            for i, h in ((0, hA), (1, hB)):
                                row0 = (h % 2) * HD
                                nc.tensor.matmul(
                                    sc[i][:, c, :],
                                    kT_sb[
                                        row0 : row0 + HD,
                                        h // 2,
                                        kc * 128 : (kc + 1) * 128,
                                    ],
                                    qT_sb[
                                        row0 : row0 + HD,
                                        h // 2,
                                        qb * QBLK : (qb + 1) * QBLK,
                                    ],
                                    start=True,
                                    stop=True,
                                    tile_position=(row0, 0),
                                )
                        pT = ppool.tile([128, 2, GRP, QBLK], F16, tag="pT", name="pT")
                        for i in range(2):
                            nc.scalar.activation(
                                pT[:, i, :gs, :],
                                sc[i][:, :gs, :],
                                mybir.ActivationFunctionType.Exp,
                            )
                        # apply 0/1 keep-mask to both heads in one DVE op
                        # (fp16 2x mode, mask broadcast over the head axis)
                        nc.vector.tensor_mul(
                            pT[:, :, :gs, :],
                            pT[:, :, :gs, :],
                            mh[:, moff : moff + gs, :].unsqueeze(1).broadcast_to(
                                [128, 2, gs, QBLK]
                            ),
                        )
                        for i, h in ((0, hA), (1, hB)):
                            for c in range(gs):
                                kc = g0 + c
                                nc.tensor.matmul(
                                    av[i][:],
                                    vaug_sb[:, kc, h * (HD + 1) : (h + 1) * (HD + 1)],
                                    pT[:, i, c, :],
                                    start=(kc == 0),
                                    stop=(kc == KC - 1),
                                )
                    nc.vector.tensor_copy(av_all[:, hA, :], av[0][:])
                    nc.scalar.copy(av_all[:, hB, :], av[1][:])

                    # normalize this head pair now (pipelines behind the next
                    # pair's matmul streak instead of serializing at qb end)
                    dstack = dpool.tile([2, QBLK], F32, tag="dstack", name="dstack")
                    nc.gpsimd.dma_start(dstack[:], av_all[HD : HD + 1, hA : hB + 1, :])
                    recip = dpool.tile([2, QBLK], F32, tag="recip", name="recip")
                    nc.vector.reciprocal_approx_fast(recip[:], dstack[:])
                    r2 = bpool.tile([1, 2, QBLK], F32, tag="r2", name="r2")
                    nc.gpsimd.dma_start(r2[:], recip[:])
                    for i, h in ((0, hA), (1, hB)):
                        bc = bpool.tile([HD, QBLK], F32, tag="bc", name="bc")
                        nc.gpsimd.partition_broadcast(bc[:], r2[:, i, :])
                        if i == 0:
                            dst = attn_sb[:HD, hp, qb * QBLK : (qb + 1) * QBLK]
                            nc.vector.tensor_mul(dst, av_all[:HD, h, :], bc[:])
                        else:
                            tmp = bpool.tile([HD, QBLK], F16, tag="tmpo", name="tmpo")
                            nc.vector.tensor_mul(tmp[:], av_all[:HD, h, :], bc[:])
                            nc.gpsimd.dma_start(
                                attn_sb[HD:128, hp, qb * QBLK : (qb + 1) * QBLK],
                                tmp[:],
                            )
                # project the previous query block while this one's softmax
                # normalization completes (keeps the PE busy, spreads y DMA)
                if qb >= 1:
                    proj(qb - 1)
            proj(QB - 1)

    nc.compile()
    return nc


def _prep_core_inputs(x, mask, w_qkv, w_proj, core):
    b, g = core // 2, core % 2
    scale = HD ** -0.5
    s0, s1 = 384 * g, 384 * (g + 1)
    return {
        "xT": np.ascontiguousarray(x[b].T).astype(np.float16),
        "wq": np.ascontiguousarray((w_qkv[s0:s1, :] * scale).T).astype(np.float16),
        "wk": np.ascontiguousarray(w_qkv[C + s0 : C + s1, :].T).astype(np.float16),
        "wv": np.ascontiguousarray(w_qkv[2 * C + s0 : 2 * C + s1, :].T).astype(np.float16),
        "wproj": np.ascontiguousarray(w_proj[:, s0:s1].T).astype(np.float16),
        "keepT": np.ascontiguousarray((1 - mask[b]).T).astype(np.float16),
        "vones": np.ones((128, S // 128 * H_PER_CORE), dtype=np.float16),
    }


_NC_CACHE = {}


def get_nc():
    if "nc" not in _NC_CACHE:
        _NC_CACHE["nc"] = _build_kernel()
    return _NC_CACHE["nc"]


def _build_runner(nc):
    """Reusable jitted shard_map callable over the 8 cores (mirrors
    bass2jax.run_bass_via_pjrt but cacheable across calls)."""
    import jax
    from jax.experimental.shard_map import shard_map
    from jax.sharding import Mesh, PartitionSpec

    from concourse.bass2jax import (
        _bass_exec_p,
        install_neuronx_cc_hook,
        partition_id_tensor,
    )

    install_neuronx_cc_hook()
    partition_name = nc.partition_id_tensor.name if nc.partition_id_tensor else None
    in_names, out_names, out_avals, zero_outs = [], [], [], []
    for alloc in nc.m.functions[0].allocations:
        if not isinstance(alloc, mybir.MemoryLocationSet):
            continue
        name = alloc.memorylocations[0].name
        if alloc.kind == "ExternalInput":
            if name != partition_name:
                in_names.append(name)
        elif alloc.kind == "ExternalOutput":
            out_names.append(name)
            shape = tuple(alloc.tensor_shape)
            dtype = mybir.dt.np(alloc.dtype)
            out_avals.append(jax.core.ShapedArray(shape, dtype))
            zero_outs.append(np.zeros(shape, dtype))
    n_params = len(in_names)
    all_in_names = list(in_names) + list(out_names)
    if partition_name is not None:
        all_in_names.append(partition_name)

    def _body(*args):
        operands = list(args)
        if partition_name is not None:
            operands.append(partition_id_tensor())
        outs = _bass_exec_p.bind(
            *operands,
            out_avals=tuple(out_avals),
            in_names=tuple(all_in_names),
            out_names=tuple(out_names),
            lowering_input_output_aliases=(),
            sim_require_finite=True,
            sim_require_nnan=True,
            nc=nc,
        )
        return tuple(outs)

    n_cores = nc.num_devices
    devices = jax.devices()[:n_cores]
    mesh = Mesh(np.asarray(devices), ("core",))
    in_specs = (PartitionSpec("core"),) * (n_params + len(out_names))
    out_specs = (PartitionSpec("core"),) * len(out_names)
    fn = jax.jit(
        shard_map(
            _body, mesh=mesh, in_specs=in_specs, out_specs=out_specs, check_rep=False
        ),
        keep_unused=True,
    )
    return fn, in_names, out_names, zero_outs


_RUNNER_CACHE = {}


def get_runner(nc, in_maps):
    """Return (fn, dev_args) for repeated dispatch of `nc` with `in_maps`."""
    import jax
    from jax.sharding import Mesh, NamedSharding, PartitionSpec

    key = id(nc)
    if key not in _RUNNER_CACHE:
        _RUNNER_CACHE[key] = _build_runner(nc)
    fn, in_names, out_names, zero_outs = _RUNNER_CACHE[key]
    n_cores = nc.num_devices
    mesh = Mesh(np.asarray(jax.devices()[:n_cores]), ("core",))
    shard = NamedSharding(mesh, PartitionSpec("core"))
    concat_in = [
        np.concatenate([np.asarray(in_maps[c][n]) for c in range(n_cores)], axis=0)
        for n in in_names
    ]
    dev_in = [jax.device_put(a, shard) for a in concat_in]
    zkey = ("zeros", key)
    if zkey not in _RUNNER_CACHE:
        concat_zeros = [
            np.zeros((n_cores * z.shape[0], *z.shape[1:]), z.dtype) for z in zero_outs
        ]
        _RUNNER_CACHE[zkey] = [jax.device_put(a, shard) for a in concat_zeros]
    return fn, dev_in + _RUNNER_CACHE[zkey]


def run_cached(nc, in_maps):
    """Execute via the cached runner; returns per-core result dicts."""
    fn, dev_args = get_runner(nc, in_maps)
    out_arrs = fn(*dev_args)
    _, _, out_names, zero_outs = _RUNNER_CACHE[id(nc)]
    n_cores = nc.num_devices
    fetched = [
        np.asarray(a).reshape(n_cores, *zero_outs[i].shape)
        for i, a in enumerate(out_arrs)
    ]
    return [
        {name: fetched[i][c] for i, name in enumerate(out_names)}
        for c in range(n_cores)
    ]


def make_in_maps(x, mask, w_qkv, w_proj):
    return [_prep_core_inputs(x, mask, w_qkv, w_proj, c) for c in range(N_CORES)]


def combine(results, b_proj):
    outs = []
    for b in range(B):
        outs.append(results[2 * b]["y"] + results[2 * b + 1]["y"] + b_proj[None, :])
    return np.stack(outs).astype(np.float32)


def kernel(x, mask, w_qkv, w_proj, b_proj):
    x = np.asarray(x, dtype=np.float32)
    mask = np.asarray(mask)
    w_qkv = np.asarray(w_qkv, dtype=np.float32)
    w_proj = np.asarray(w_proj, dtype=np.float32)
    b_proj = np.asarray(b_proj, dtype=np.float32)

    nc = get_nc()
    in_maps = make_in_maps(x, mask, w_qkv, w_proj)
    try:
        results = run_cached(nc, in_maps)
    except Exception:
        results = run_bass_kernel_spmd(nc, in_maps, list(range(N_CORES))).results
    return combine(results, b_proj)


# revision 15
# speedup vs baseline: 1.0120x; 1.0073x over previous
"""TRN2 Bass kernel: masked multi-head attention block (B=4, S=2048, C=768, H=12).

Sharding: 8 cores = 4 batches x 2 head-groups (6 heads each).  Each core runs a
flash-attention-style Bass/Tile kernel over its (batch, head-group) shard.

v2: full-fp16 matmul datapath (fp16 streams at 1 cyc/col on the PE like bf16,
enables fast-weight-load, and halves DVE/DMA cost vs fp32) with the mask
applied as a post-exp elementwise multiply on the vector engine (fp16 2x mode)
instead of a pre-exp fp32 add from PSUM (1x mode):

  qT/kT: [384, S] feature-major fp16 projections from xT (q pre-scaled)
  v:     [S, 6*65] fp16, a ones column appended per head (softmax denominator)
  scoresT[k, q] = k . q (contract hd=64, head pairs row-packed on the PE array)
  pT = exp(scoresT) on the scalar engine (psum -> sbuf fp16)
  pT *= keepT (0/1 mask) on the vector engine (fp16 2x)
  avT[65, 512] accumulated over key chunks; row 64 = softmax denominator
  attn_outT = avT[0:64] * recip(denominator)  (partition-broadcast on gpsimd)
  y_partial = attn_outT.T @ w_projT slice  (row-parallel output projection)

Host-side: transposes/slices the weights per core into fp16, converts the mask
to a 0/1 fp16 keep-mask, sums the two per-batch partials, and adds b_proj.
"""

from contextlib import ExitStack

import numpy as np

import concourse.tile as tile
from concourse import bacc, mybir
from concourse.bass_utils import run_bass_kernel_spmd

F32 = mybir.dt.float32
F16 = mybir.dt.float16
I16 = mybir.dt.int16

# fp16 Schraudolph exp: fp16_bits(exp(s)) ~= round(s*1024*log2(e) + B).
# B = 15*1024 - 59 centers the (1+x)/2^x mantissa error (rms 1.8%, max 3.9%);
# used on a small fraction of groups to offload the scalar engine.
SCHRAU_A = 1477.3195458351
SCHRAU_B = 15301.0

B, S, C, H = 4, 2048, 768, 12
HD = 64
H_PER_CORE = 6
D_CORE = H_PER_CORE * HD  # 384
QBLK = 512
GRP = 2
N_CORES = 8


def _build_kernel():
    nc = bacc.Bacc(
        trn_type="TRN2", target_bir_lowering=False, debug=False, num_devices=N_CORES
    )
    KC = S // 128  # 16 key chunks
    QB = S // QBLK  # 4 query blocks
    NB = S // QBLK  # 4 column chunks for projections
    ST = S // 128  # 16 sequence tiles
    groups = [(2 * i, GRP) for i in range(KC // GRP)]  # 8 groups of 2 key chunks

    xT = nc.dram_tensor("xT", [C, S], F16, kind="ExternalInput").ap()
    wq = nc.dram_tensor("wq", [C, D_CORE], F16, kind="ExternalInput").ap()
    wk = nc.dram_tensor("wk", [C, D_CORE], F16, kind="ExternalInput").ap()
    wv = nc.dram_tensor("wv", [C, D_CORE], F16, kind="ExternalInput").ap()
    wproj = nc.dram_tensor("wproj", [D_CORE, C], F16, kind="ExternalInput").ap()
    vones = nc.dram_tensor("vones", [128, S // 128 * H_PER_CORE], F16, kind="ExternalInput").ap()
    keepT = nc.dram_tensor("keepT", [S, S], F16, kind="ExternalInput").ap()
    y = nc.dram_tensor("y", [S, C], F32, kind="ExternalOutput").ap()

    with tile.TileContext(nc) as tc, ExitStack() as ctx:
        consts = ctx.enter_context(tc.tile_pool(name="consts", bufs=1))
        qkv_pool = ctx.enter_context(tc.tile_pool(name="qkv", bufs=1))
        mpool = ctx.enter_context(tc.tile_pool(name="mask", bufs=4))
        ypool = ctx.enter_context(tc.tile_pool(name="y", bufs=3))

        keepT_r = keepT.rearrange("(kc p) q -> p kc q", p=128)
        mask_cache = {}

        def load_mask(qb_i):
            halves = []
            for half in range(KC // 8):
                mh = mpool.tile([128, 8, QBLK], F16, tag="mask", name="mask_h")
                nc.sync.dma_start(
                    mh[:],
                    keepT_r[
                        :,
                        half * 8 : (half + 1) * 8,
                        qb_i * QBLK : (qb_i + 1) * QBLK,
                    ],
                )
                halves.append(mh)
            return halves

        qT_sb = qkv_pool.tile([128, 3, S], F16)
        kT_sb = qkv_pool.tile([128, 3, S], F16)
        vaug_sb = qkv_pool.tile([128, ST, H_PER_CORE * (HD + 1)], F16)
        attn_sb = qkv_pool.tile([128, 3, S], F16)

        # ---------------- phase 1: qkv projections ----------------
        with ExitStack() as p1:
            wpool = p1.enter_context(tc.tile_pool(name="w1", bufs=1))
            xpool = p1.enter_context(tc.tile_pool(name="x1", bufs=1))
            ps1 = p1.enter_context(tc.tile_pool(name="ps1", bufs=3, space="PSUM"))
            psv1 = p1.enter_context(tc.tile_pool(name="psv1", bufs=2, space="PSUM"))

            wk_sb = wpool.tile([128, 6, D_CORE], F16)
            wv_sb = wpool.tile([128, 6, D_CORE], F16)
            wq_sb = wpool.tile([128, 6, D_CORE], F16)
            xT_sb = xpool.tile([128, 6, S], F16)
            xT_r = xT.rearrange("(t p) s -> p t s", p=128)

            def x_chunk_dma(nb):
                nc.sync.dma_start(
                    xT_sb[:, :, nb * QBLK : (nb + 1) * QBLK],
                    xT_r[:, :, nb * QBLK : (nb + 1) * QBLK],
                )

            # interleave weight and x-chunk DMAs in consumption order so the
            # first projection matmuls start as early as possible
            nc.sync.dma_start(wk_sb[:], wk.rearrange("(t p) d -> p t d", p=128))
            x_chunk_dma(0)
            nc.sync.dma_start(wv_sb[:], wv.rearrange("(t p) d -> p t d", p=128))
            x_chunk_dma(1)
            x_chunk_dma(2)
            x_chunk_dma(3)
            nc.sync.dma_start(wq_sb[:], wq.rearrange("(t p) d -> p t d", p=128))

            # warm the ACT exp table during input DMA so the first real exp
            # doesn't pay the ~2.7us table-load
            dummy = consts.tile([1, 8], F32)
            nc.scalar.activation(dummy[:], wk_sb[0:1, 0, 0:8], mybir.ActivationFunctionType.Exp)

            # non-critical input DMAs after the phase-1 ones
            wproj_sb = consts.tile([128, 3, C], F16)
            nc.sync.dma_start(wproj_sb[:], wproj.rearrange("(t p) o -> p t o", p=128))
            # ones columns (softmax denominator) come from DRAM
            vaug_ones = vaug_sb.rearrange("p st (h u) -> p st h u", u=HD + 1)[:, :, :, HD]
            nc.sync.dma_start(
                vaug_ones, vones.rearrange("p (st h) -> p st h", h=H_PER_CORE)
            )
            mask_cache[0] = load_mask(0)
            mask_cache[1] = load_mask(1)

            # k projection first (phase 2 needs all of kT), then v, then q
            def qk_proj(w_sb, dst):
                for nb in range(NB):
                    for m in range(3):
                        ps = ps1.tile([128, QBLK], F32, tag="psqk", name="psqk")
                        for k in range(6):
                            nc.tensor.matmul(
                                ps[:],
                                w_sb[:, k, m * 128 : (m + 1) * 128],
                                xT_sb[:, k, nb * QBLK : (nb + 1) * QBLK],
                                start=(k == 0),
                                stop=(k == 5),
                            )
                        nc.vector.tensor_copy(
                            dst[:, m, nb * QBLK : (nb + 1) * QBLK], ps[:]
                        )

            qk_proj(wk_sb, kT_sb)
            for st in range(ST):
                psv = psv1.tile([128, D_CORE], F32, tag="psv", name="psv")
                for k in range(6):
                    nc.tensor.matmul(
                        psv[:],
                        xT_sb[:, k, st * 128 : (st + 1) * 128],
                        wv_sb[:, k, :],
                        start=(k == 0),
                        stop=(k == 5),
                    )
                # scatter the 6 heads into the augmented-v layout in one op
                nc.vector.tensor_copy(
                    vaug_sb.rearrange("p st (h u) -> p st h u", u=HD + 1)[
                        :, st, :, 0:HD
                    ],
                    psv.rearrange("p (h d) -> p h d", d=HD)[:],
                )
            qk_proj(wq_sb, qT_sb)

        # ---------------- phase 2: attention (+ interleaved projection) ----
        with ExitStack() as p2:
            ppool = p2.enter_context(tc.tile_pool(name="pT", bufs=5))
            dpool = p2.enter_context(tc.tile_pool(name="div", bufs=2))
            bpool = p2.enter_context(tc.tile_pool(name="bcast", bufs=2))
            apool = p2.enter_context(tc.tile_pool(name="avsb", bufs=1))
            ps_s = p2.enter_context(tc.tile_pool(name="ps_s", bufs=3, space="PSUM"))
            ps_av = p2.enter_context(tc.tile_pool(name="ps_av", bufs=2, space="PSUM"))

            y_r = y.rearrange("(st p) o -> st p o", p=128)

            def proj(qb):
                # output projection for the 4 sequence tiles of query block qb
                for st in range(4 * qb, 4 * qb + 4):
                    y_sb = ypool.tile([128, C], F32, tag="ysb", name="y_sb")
                    for nb2 in range(2):
                        ps = ps_av.tile([128, 384], F32, tag="av", name="psy")
                        for k3 in range(3):
                            nc.tensor.matmul(
                                ps[:],
                                attn_sb[:, k3, st * 128 : (st + 1) * 128],
                                wproj_sb[:, k3, nb2 * 384 : (nb2 + 1) * 384],
                                start=(k3 == 0),
                                stop=(k3 == 2),
                            )
                        if nb2 == 0:
                            nc.vector.tensor_copy(y_sb[:, :384], ps[:])
                        else:
                            nc.scalar.copy(y_sb[:, 384:], ps[:])
                    nc.sync.dma_start(y_r[st], y_sb[:])

            for qb in range(QB):
                if qb + 2 < QB:
                    mask_cache[qb + 2] = load_mask(qb + 2)
                mask_halves = mask_cache.pop(qb)
                av_all = apool.tile([HD + 1, H_PER_CORE, QBLK], F32, tag="av_all", name="av_all")

                for hp in range(3):
                    hA, hB = 2 * hp, 2 * hp + 1
                    av = [
                        ps_av.tile([HD + 1, QBLK], F32, tag="av", name=f"av{hp}a"),
                        ps_av.tile([HD + 1, QBLK], F32, tag="av", name=f"av{hp}b"),
                    ]
                    for (g0, gs) in groups:
                        mh = mask_halves[g0 // 8]
                        moff = g0 % 8
                        sc = [
                            ps_s.tile([128, GRP, QBLK], F32, tag="sc", name="scA"),
                            ps_s.tile([128, GRP, QBLK], F32, tag="sc", name="scB"),
                        ]
                        # interleave heads so the row-packed matmul pairs
                        # overlap on the PE array (different row groups)
                        for c in range(gs):
                            kc = g0 + c
                            for i, h in ((0, hA), (1, hB)):
                                row0 = (h % 2) * HD
                                nc.tensor.matmul(
                                    sc[i][:, c, :],
                                    kT_sb[
                                        row0 : row0 + HD,
                                        h // 2,
                                        kc * 128 : (kc + 1) * 128,
                                    ],
                                    qT_sb[
                                        row0 : row0 + HD,
                                        h // 2,
                                        qb * QBLK : (qb + 1) * QBLK,
                                    ],
                                    start=True,
                                    stop=True,
                                    tile_position=(row0, 0),
                                )
                        pT = ppool.tile([128, 2, GRP, QBLK], F16, tag="pT", name="pT")
                        for i in range(2):
                            nc.scalar.activation(
                                pT[:, i, :gs, :],
                                sc[i][:, :gs, :],
                                mybir.ActivationFunctionType.Exp,
                            )
                        # apply 0/1 keep-mask to both heads in one DVE op
                        # (fp16 2x mode, mask broadcast over the head axis)
                        nc.vector.tensor_mul(
                            pT[:, :, :gs, :],
                            pT[:, :, :gs, :],
                            mh[:, moff : moff + gs, :].unsqueeze(1).broadcast_to(
                                [128, 2, gs, QBLK]
                            ),
                        )
                        for i, h in ((0, hA), (1, hB)):
                            for c in range(gs):
                                kc = g0 + c
                                nc.tensor.matmul(
                                    av[i][:],
                                    vaug_sb[:, kc, h * (HD + 1) : (h + 1) * (HD + 1)],
                                    pT[:, i, c, :],
                                    start=(kc == 0),
                                    stop=(kc == KC - 1),
                                )
                    nc.vector.tensor_copy(av_all[:, hA, :], av[0][:])
                    nc.scalar.copy(av_all[:, hB, :], av[1][:])

                    # normalize this head pair now (pipelines behind the next
                    # pair's matmul streak instead of serializing at qb end)
                    dstack = dpool.tile([2, QBLK], F32, tag="dstack", name="dstack")
                    nc.gpsimd.dma_start(dstack[:], av_all[HD : HD + 1, hA : hB + 1, :])
                    recip = dpool.tile([2, QBLK], F32, tag="recip", name="recip")
                    nc.vector.reciprocal_approx_fast(recip[:], dstack[:])
                    r2 = bpool.tile([1, 2, QBLK], F32, tag="r2", name="r2")
                    nc.gpsimd.dma_start(r2[:], recip[:])
                    # odd head first: its broadcast->mult->partition-shift
                    # DMA is the longest chain and the last attn_sb writer
                    # gating the projection matmuls
                    for i, h in ((1, hB), (0, hA)):
                        bc = bpool.tile([HD, QBLK], F32, tag="bc", name="bc")
                        nc.gpsimd.partition_broadcast(bc[:], r2[:, i, :])
                        if i == 0:
                            dst = attn_sb[:HD, hp, qb * QBLK : (qb + 1) * QBLK]
                            nc.vector.tensor_mul(dst, av_all[:HD, h, :], bc[:])
                        else:
                            tmp = bpool.tile([HD, QBLK], F16, tag="tmpo", name="tmpo")
                            nc.vector.tensor_mul(tmp[:], av_all[:HD, h, :], bc[:])
                            nc.gpsimd.dma_start(
                                attn_sb[HD:128, hp, qb * QBLK : (qb + 1) * QBLK],
                                tmp[:],
                            )
                # project the previous query block while this one's softmax
                # normalization completes (keeps the PE busy, spreads y DMA)
                if qb >= 1:
                    proj(qb - 1)
            proj(QB - 1)

    nc.compile()
    return nc


def _prep_core_inputs(x, mask, w_qkv, w_proj, core):
    b, g = core // 2, core % 2
    scale = HD ** -0.5
    s0, s1 = 384 * g, 384 * (g + 1)
    return {
        "xT": np.ascontiguousarray(x[b].T).astype(np.float16),
        "wq": np.ascontiguousarray((w_qkv[s0:s1, :] * scale).T).astype(np.float16),
        "wk": np.ascontiguousarray(w_qkv[C + s0 : C + s1, :].T).astype(np.float16),
        "wv": np.ascontiguousarray(w_qkv[2 * C + s0 : 2 * C + s1, :].T).astype(np.float16),
        "wproj": np.ascontiguousarray(w_proj[:, s0:s1].T).astype(np.float16),
        "keepT": np.ascontiguousarray((1 - mask[b]).T).astype(np.float16),
        "vones": np.ones((128, S // 128 * H_PER_CORE), dtype=np.float16),
    }


_NC_CACHE = {}


def get_nc():
    if "nc" not in _NC_CACHE:
        _NC_CACHE["nc"] = _build_kernel()
    return _NC_CACHE["nc"]


def _build_runner(nc):
    """Reusable jitted shard_map callable over the 8 cores (mirrors
    bass2jax.run_bass_via_pjrt but cacheable across calls)."""
    import jax
    from jax.experimental.shard_map import shard_map
    from jax.sharding import Mesh, PartitionSpec

    from concourse.bass2jax import (
        _bass_exec_p,
        install_neuronx_cc_hook,
        partition_id_tensor,
    )

    install_neuronx_cc_hook()
    partition_name = nc.partition_id_tensor.name if nc.partition_id_tensor else None
    in_names, out_names, out_avals, zero_outs = [], [], [], []
    for alloc in nc.m.functions[0].allocations:
        if not isinstance(alloc, mybir.MemoryLocationSet):
            continue
        name = alloc.memorylocations[0].name
        if alloc.kind == "ExternalInput":
            if name != partition_name:
                in_names.append(name)
        elif alloc.kind == "ExternalOutput":
            out_names.append(name)
            shape = tuple(alloc.tensor_shape)
            dtype = mybir.dt.np(alloc.dtype)
            out_avals.append(jax.core.ShapedArray(shape, dtype))
            zero_outs.append(np.zeros(shape, dtype))
    n_params = len(in_names)
    all_in_names = list(in_names) + list(out_names)
    if partition_name is not None:
        all_in_names.append(partition_name)

    def _body(*args):
        operands = list(args)
        if partition_name is not None:
            operands.append(partition_id_tensor())
        outs = _bass_exec_p.bind(
            *operands,
            out_avals=tuple(out_avals),
            in_names=tuple(all_in_names),
            out_names=tuple(out_names),
            lowering_input_output_aliases=(),
            sim_require_finite=True,
            sim_require_nnan=True,
            nc=nc,
        )
        return tuple(outs)

    n_cores = nc.num_devices
    devices = jax.devices()[:n_cores]
    mesh = Mesh(np.asarray(devices), ("core",))
    in_specs = (PartitionSpec("core"),) * (n_params + len(out_names))
    out_specs = (PartitionSpec("core"),) * len(out_names)
    fn = jax.jit(
        shard_map(
            _body, mesh=mesh, in_specs=in_specs, out_specs=out_specs, check_rep=False
        ),
        keep_unused=True,
    )
    return fn, in_names, out_names, zero_outs


_RUNNER_CACHE = {}


def get_runner(nc, in_maps):
    """Return (fn, dev_args) for repeated dispatch of `nc` with `in_maps`."""
    import jax
    from jax.sharding import Mesh, NamedSharding, PartitionSpec

    key = id(nc)
    if key not in _RUNNER_CACHE:
        _RUNNER_CACHE[key] = _build_runner(nc)
    fn, in_names, out_names, zero_outs = _RUNNER_CACHE[key]
    n_cores = nc.num_devices
    mesh = Mesh(np.asarray(jax.devices()[:n_cores]), ("core",))
    shard = NamedSharding(mesh, PartitionSpec("core"))
    concat_in = [
        np.concatenate([np.asarray(in_maps[c][n]) for c in range(n_cores)], axis=0)
        for n in in_names
    ]
    dev_in = [jax.device_put(a, shard) for a in concat_in]
    zkey = ("zeros", key)
    if zkey not in _RUNNER_CACHE:
        concat_zeros = [
            np.zeros((n_cores * z.shape[0], *z.shape[1:]), z.dtype) for z in zero_outs
        ]
        _RUNNER_CACHE[zkey] = [jax.device_put(a, shard) for a in concat_zeros]
    return fn, dev_in + _RUNNER_CACHE[zkey]


def run_cached(nc, in_maps):
    """Execute via the cached runner; returns per-core result dicts."""
    fn, dev_args = get_runner(nc, in_maps)
    out_arrs = fn(*dev_args)
    _, _, out_names, zero_outs = _RUNNER_CACHE[id(nc)]
    n_cores = nc.num_devices
    fetched = [
        np.asarray(a).reshape(n_cores, *zero_outs[i].shape)
        for i, a in enumerate(out_arrs)
    ]
    return [
        {name: fetched[i][c] for i, name in enumerate(out_names)}
        for c in range(n_cores)
    ]


def make_in_maps(x, mask, w_qkv, w_proj):
    return [_prep_core_inputs(x, mask, w_qkv, w_proj, c) for c in range(N_CORES)]


def combine(results, b_proj):
    outs = []
    for b in range(B):
        outs.append(results[2 * b]["y"] + results[2 * b + 1]["y"] + b_proj[None, :])
    return np.stack(outs).astype(np.float32)


def kernel(x, mask, w_qkv, w_proj, b_proj):
    x = np.asarray(x, dtype=np.float32)
    mask = np.asarray(mask)
    w_qkv = np.asarray(w_qkv, dtype=np.float32)
    w_proj = np.asarray(w_proj, dtype=np.float32)
    b_proj = np.asarray(b_proj, dtype=np.float32)

    nc = get_nc()
    in_maps = make_in_maps(x, mask, w_qkv, w_proj)
    try:
        results = run_cached(nc, in_maps)
    except Exception:
        results = run_bass_kernel_spmd(nc, in_maps, list(range(N_CORES))).results
    return combine(results, b_proj)
